# revision 1
# baseline (speedup 1.0000x reference)
"""Fused linear + cross-entropy loss (BaseChunkLoss) on 8 trn2 NeuronCores.

Strategy (per the sharding hint: token/data parallel):
  - Tokens (N=8192) are sharded 8 ways: each core handles 1024 tokens x the
    full vocab (32000), so every core computes a complete logsumexp for its
    tokens and no cross-device reduction of partials is needed.
  - head_weight streams through each core (262 MB fp32 -> ~360 GB/s DMA,
    overlapped with compute); the 1024-token hidden slice stays resident in
    SBUF.
  - The final tiny reduction - log(s), nll = lse - tgt, weighted mean, and
    the 8-way scalar combine - happens on host, standing in for the
    wrapper's all_reduce of the scalar loss.

Device kernel layout: tokens on PSUM partitions, vocab on the free dim.
  stationary lhsT = hidden^T tile [128 d x 128 tok]
  moving rhs      = weight^T tile [128 d x 500 vocab]
  psum [128 tok x 500 vocab] fp32, accumulated over the D=2048 contraction.
Matmuls run in fp8e4m3 with perf_mode=DoubleRow (2 contraction rows per PE
cell, K=256 per instruction; weights pre-scaled by 64 on-chip for e4m3
range, descaled during the bias add). Per 1500-wide vocab group: DVE does
(psum/64 + bias) in place, DVE extracts the target logit via
(iota == label) * logit with a fused row-sum accumulator, and ACT computes
exp with a fused row-sum accumulator. Set USE_FP8 = False for a bf16
variant (~2.5e-6 loss error instead of ~5e-5, ~1.7x slower).

Host-side input prep is layout-only (transpose/slice/cast of index arrays);
all FLOPs over hidden/weights happen on device inside the measured kernel.
"""
import numpy as np
from contextlib import ExitStack

from concourse import bacc, mybir, tile
from concourse.bass_utils import run_bass_kernel_spmd

F32 = mybir.dt.float32
BF16 = mybir.dt.bfloat16
FP8 = mybir.dt.float8e4
Alu = mybir.AluOpType
Act = mybir.ActivationFunctionType

USE_FP8 = True

N_CORES = 8
N_TOK = 8192
D = 2048
V = 32000
P = 128
KT = D // P            # 16 k-tiles of 128
BANK = 500             # vocab columns per psum bank (<= 512 fp32)
BPG = 3                # banks per vocab group
T = N_TOK // N_CORES   # 1024 tokens per core
T_CONST = T
V_CONST = V
MB = T // P            # 8 token blocks per core

W_SCALE = 64.0         # fp8 weight pre-scale (e4m3 range)
WPAD = 1536            # fp8 W tile inner stride (multiple of 16 for DoubleRow)


def _vocab_groups():
    nbanks = V // BANK
    groups = []
    b = 0
    while b < nbanks:
        nb = min(BPG, nbanks - b)
        groups.append((b * BANK, nb * BANK, nb, b))
        b += nb
    return groups


def _declare_io(nc):
    # h and W arrive pre-transposed from host: h [D, T], W [D, V]
    return (
        nc.declare_dram_parameter("h", [D, T], F32, isOutput=False),
        nc.declare_dram_parameter("W", [D, V], F32, isOutput=False),
        nc.declare_dram_parameter("bias", [V], F32, isOutput=False),
        nc.declare_dram_parameter("iota", [V], F32, isOutput=False),
        nc.declare_dram_parameter("labs", [P, MB], F32, isOutput=False),
        nc.declare_dram_parameter("s_out", [P, MB], F32, isOutput=True),
        nc.declare_dram_parameter("t_out", [P, MB], F32, isOutput=True),
    )


def _postops(nc, pt, nb, nv, bb, ii, labs_t, m, col, junk, ejunk,
             s_cols, t_cols, descale):
    psl = pt[:, 0:nb, 0:BANK]
    bbv = bb[:, 0:nv].rearrange("p (b c) -> p b c", c=BANK)
    iiv = ii[:, 0:nv].rearrange("p (b c) -> p b c", c=BANK)
    if descale:
        nc.vector.scalar_tensor_tensor(
            psl, psl, 1.0 / W_SCALE, bbv, op0=Alu.mult, op1=Alu.add)
    else:
        nc.vector.tensor_tensor(psl, psl, bbv, op=Alu.add)
    jt = junk.tile([P, BPG, BANK], F32, tag="junk")
    nc.vector.scalar_tensor_tensor(
        jt[:, 0:nb, :], iiv, labs_t[:, m:m + 1], psl,
        op0=Alu.is_equal, op1=Alu.mult,
        accum_out=t_cols[:, col:col + 1],
    )
    et = ejunk.tile([P, BPG, BANK], F32, tag="ejunk")
    nc.scalar.activation(
        et[:, 0:nb, :], psl, Act.Exp, accum_out=s_cols[:, col:col + 1])


def _finish(nc, acc, s_cols, t_cols, ng, s_out, t_out):
    s_fin = acc.tile([P, MB], F32, tag="sfin")
    t_fin = acc.tile([P, MB], F32, tag="tfin")
    for m in range(MB):
        nc.vector.tensor_reduce(
            s_fin[:, m:m + 1], s_cols[:, m * ng:(m + 1) * ng],
            axis=mybir.AxisListType.X, op=Alu.add)
        nc.vector.tensor_reduce(
            t_fin[:, m:m + 1], t_cols[:, m * ng:(m + 1) * ng],
            axis=mybir.AxisListType.X, op=Alu.add)
    nc.sync.dma_start(s_out[:], s_fin[:])
    nc.sync.dma_start(t_out[:], t_fin[:])


def _build_bf16():
    groups = _vocab_groups()
    ng = len(groups)
    nc = bacc.Bacc("TRN2", target_bir_lowering=False, debug=False)
    h_d, W_d, bias_d, iota_d, labs_d, s_out, t_out = _declare_io(nc)
    W_r = W_d[:].rearrange("(ko ki) v -> ko ki v", ki=P)   # [KT, 128, V]
    h_r = h_d[:].rearrange("(ko ki) t -> ko ki t", ki=P)   # [KT, 128, T]

    with tile.TileContext(nc) as tc, ExitStack() as ctx:
        hpool = ctx.enter_context(tc.tile_pool(name="hT", bufs=1))
        hstage = ctx.enter_context(tc.tile_pool(name="hstage", bufs=2))
        wpool = ctx.enter_context(tc.tile_pool(name="w", bufs=3))
        wstage = ctx.enter_context(tc.tile_pool(name="wstage", bufs=2))
        bpool = ctx.enter_context(tc.tile_pool(name="bias", bufs=2))
        ipool = ctx.enter_context(tc.tile_pool(name="iota", bufs=2))
        pspool = ctx.enter_context(tc.tile_pool(name="ps", bufs=2, space="PSUM"))
        junk = ctx.enter_context(tc.tile_pool(name="junk", bufs=2))
        ejunk = ctx.enter_context(tc.tile_pool(name="ejunk", bufs=2))
        acc = ctx.enter_context(tc.tile_pool(name="acc", bufs=1))

        labs_t = acc.tile([P, MB], F32, tag="labs")
        nc.sync.dma_start(labs_t[:], labs_d[:])
        s_cols = acc.tile([P, MB * ng], F32, tag="scols")
        t_cols = acc.tile([P, MB * ng], F32, tag="tcols")

        hT = hpool.tile([P, KT, T], BF16, tag="hT")
        for k in range(KT):
            st = hstage.tile([P, T], F32, tag="hstage")
            nc.sync.dma_start(st[:], h_r[k])
            nc.vector.tensor_copy(hT[:, k, :], st[:])

        for voff, nv, nb, col0 in groups:
            wv = wpool.tile([P, KT, BPG * BANK], BF16, tag="w")
            for k in range(KT):
                ws = wstage.tile([P, BPG * BANK], F32, tag="wstage")
                nc.sync.dma_start(ws[:, :nv], W_r[k, :, voff:voff + nv])
                nc.scalar.copy(wv[:, k, :nv], ws[:, :nv])
            bb = bpool.tile([P, BPG * BANK], F32, tag="bias")
            nc.scalar.dma_start(
                bb[:, :nv], bias_d[voff:voff + nv].partition_broadcast(P))
            ii = ipool.tile([P, BPG * BANK], F32, tag="iota")
            nc.scalar.dma_start(
                ii[:, :nv], iota_d[voff:voff + nv].partition_broadcast(P))

            for m in range(MB):
                pt = pspool.tile([P, BPG, 512], F32, tag="ps")
                for k in range(KT):
                    lhsT = hT[:, k, m * P:(m + 1) * P]
                    for bk in range(nb):
                        nc.tensor.matmul(
                            pt[:, bk, 0:BANK], lhsT,
                            wv[:, k, bk * BANK:(bk + 1) * BANK],
                            start=(k == 0), stop=(k == KT - 1),
                        )
                col = m * ng + (col0 // BPG)
                _postops(nc, pt, nb, nv, bb, ii, labs_t, m, col, junk, ejunk,
                         s_cols, t_cols, descale=False)

        _finish(nc, acc, s_cols, t_cols, ng, s_out, t_out)

    nc.compile()
    return nc


def _build_fp8():
    T, V = T_CONST, V_CONST
    """fp8 DoubleRow v5: 4 banks/group; tgt via exact f32 rowdot of gathered
    weight rows (host gathers W[labels]; device does the dot); drain chain is
    one DVE op + one ACT op per psum slot."""
    BPG4 = 4
    GV = BPG4 * BANK            # 2000 vocab per group
    WPAD4 = 2048
    assert V % GV == 0
    MB = T // P
    ng = V // GV
    KP2 = KT // 2

    nc = bacc.Bacc("TRN2", target_bir_lowering=False, debug=False)
    h_d = nc.declare_dram_parameter("h", [D, T], F32, isOutput=False)
    W_d = nc.declare_dram_parameter("W", [D, V], F32, isOutput=False)
    bias_d = nc.declare_dram_parameter("bias", [V], F32, isOutput=False)
    hn_d = nc.declare_dram_parameter("hn", [T, D], F32, isOutput=False)
    wg_d = nc.declare_dram_parameter("wg", [T, D], F32, isOutput=False)
    s_out = nc.declare_dram_parameter("s_out", [P, MB], F32, isOutput=True)
    t_out = nc.declare_dram_parameter("t_out", [P, MB], F32, isOutput=True)

    W_r2 = W_d[:].rearrange("(kp j ki) v -> kp ki j v", ki=P, j=2)
    h_r2 = h_d[:].rearrange("(kp j ki) t -> kp ki j t", ki=P, j=2)

    with tile.TileContext(nc) as tc, ExitStack() as ctx:
        hpool = ctx.enter_context(tc.tile_pool(name="hT", bufs=1))
        hstage = ctx.enter_context(tc.tile_pool(name="hstage", bufs=2))
        wpool = ctx.enter_context(tc.tile_pool(name="w", bufs=2))
        wstage = ctx.enter_context(tc.tile_pool(name="wstage", bufs=2))
        bpool = ctx.enter_context(tc.tile_pool(name="bias", bufs=2))
        gpool = ctx.enter_context(tc.tile_pool(name="gath", bufs=2))
        pspool = ctx.enter_context(tc.tile_pool(name="ps", bufs=2, space="PSUM"))
        ejunk = ctx.enter_context(tc.tile_pool(name="ejunk", bufs=1))
        djunk = ctx.enter_context(tc.tile_pool(name="djunk", bufs=1))
        acc = ctx.enter_context(tc.tile_pool(name="acc", bufs=1))

        s_cols = acc.tile([P, MB * ng], F32, tag="scols")
        t_fin = acc.tile([P, MB], F32, tag="tfin")

        # exact-f32 target logit: per m-block rowdot of hn and gathered rows
        for m in range(MB):
            hg = gpool.tile([P, D], F32, tag="hg")
            nc.scalar.dma_start(hg[:], hn_d[m * P:(m + 1) * P, :])
            wgt = gpool.tile([P, D], F32, tag="wgt")
            nc.scalar.dma_start(wgt[:], wg_d[m * P:(m + 1) * P, :])
            dj = djunk.tile([P, D], F32, tag="djunk")
            nc.vector.tensor_mul(dj[:], hg[:], wgt[:])
            nc.vector.tensor_reduce(
                t_fin[:, m:m + 1], dj[:], axis=mybir.AxisListType.X, op=Alu.add)

        hT = hpool.tile([P, KP2, 2, T], FP8, tag="hT")
        for kp in range(KP2):
            st = hstage.tile([P, 2, T], F32, tag="hstage")
            nc.sync.dma_start(st[:], h_r2[kp])
            nc.vector.tensor_copy(hT[:, kp, :, :], st[:])

        for g in range(ng):
            voff = g * GV
            wv = wpool.tile([P, KP2, 2, WPAD4], FP8, tag="w")
            for kp in range(KP2):
                ws = wstage.tile([P, 2, GV], F32, tag="wstage")
                nc.sync.dma_start(ws[:], W_r2[kp][:, :, voff:voff + GV])
                if kp % 2 == 0:
                    nc.scalar.mul(wv[:, kp, :, 0:GV], ws[:], W_SCALE)
                else:
                    nc.vector.tensor_scalar_mul(wv[:, kp, :, 0:GV], ws[:], W_SCALE)
            bb = bpool.tile([P, GV], F32, tag="bias")
            nc.scalar.dma_start(bb[:], bias_d[voff:voff + GV].partition_broadcast(P))

            for m in range(MB):
                pt = pspool.tile([P, BPG4, 512], F32, tag="ps")
                for kp in range(KP2):
                    lhsT = hT[:, kp, :, m * P:(m + 1) * P]
                    for bk in range(BPG4):
                        nc.tensor.matmul(
                            pt[:, bk, 0:BANK], lhsT,
                            wv[:, kp, :, bk * BANK:(bk + 1) * BANK],
                            start=(kp == 0), stop=(kp == KP2 - 1),
                            perf_mode=mybir.MatmulPerfMode.DoubleRow,
                        )
                col = m * ng + g
                psl = pt[:, 0:BPG4, 0:BANK]
                bbv = bb[:].rearrange("p (b c) -> p b c", c=BANK)
                nc.vector.scalar_tensor_tensor(
                    psl, psl, 1.0 / W_SCALE, bbv, op0=Alu.mult, op1=Alu.add)
                et = ejunk.tile([P, BPG4, BANK], F32, tag="ejunk")
                nc.scalar.activation(
                    et[:], psl, Act.Exp, accum_out=s_cols[:, col:col + 1])

        s_fin = acc.tile([P, MB], F32, tag="sfin")
        for m in range(MB):
            nc.vector.tensor_reduce(
                s_fin[:, m:m + 1], s_cols[:, m * ng:(m + 1) * ng],
                axis=mybir.AxisListType.X, op=Alu.add)
        nc.sync.dma_start(s_out[:], s_fin[:])
        nc.sync.dma_start(t_out[:], t_fin[:])

    nc.compile()
    return nc


_NC_CACHE = {}


def _get_program():
    key = "fp8" if USE_FP8 else "bf16"
    if key not in _NC_CACHE:
        _NC_CACHE[key] = _build_fp8() if USE_FP8 else _build_bf16()
    return _NC_CACHE[key]


def kernel(hidden_states, head_weight, head_bias, loss_weight, labels,
           chunk_size=None, **_unused):
    hidden = np.asarray(hidden_states, dtype=np.float32)
    W = np.asarray(head_weight, dtype=np.float32)
    bias = np.asarray(head_bias, dtype=np.float32)
    lw = np.asarray(loss_weight, dtype=np.float32)
    labels = np.asarray(labels)

    assert hidden.shape == (N_TOK, D) and W.shape == (V, D)

    nc = _get_program()
    Wt = np.ascontiguousarray(W.T)                 # [D, V]
    ht = np.ascontiguousarray(hidden.T)            # [D, N]
    in_maps = []
    if USE_FP8:
        Wg = W[labels.astype(np.int64)]            # gathered rows [N, D]
        for c in range(N_CORES):
            sl = slice(c * T, (c + 1) * T)
            in_maps.append(dict(
                h=np.ascontiguousarray(ht[:, sl]), W=Wt, bias=bias,
                hn=np.ascontiguousarray(hidden[sl]),
                wg=np.ascontiguousarray(Wg[sl])))
    else:
        iota = np.arange(V, dtype=np.float32)
        for c in range(N_CORES):
            sl = slice(c * T, (c + 1) * T)
            labs = labels[sl].reshape(MB, P).T.astype(np.float32).copy()
            in_maps.append(dict(h=np.ascontiguousarray(ht[:, sl]), W=Wt,
                                bias=bias, iota=iota, labs=labs))
    res = run_bass_kernel_spmd(nc, in_maps, list(range(N_CORES)))

    # unshard + host-side scalar combine (the "all_reduce" of the hint)
    s = np.concatenate([r["s_out"].T.reshape(-1) for r in res.results])
    tgt = np.concatenate([r["t_out"].T.reshape(-1) for r in res.results])
    if USE_FP8:
        # device produced the exact f32 dot h.W[label]; add the bias here
        tgt = tgt + bias[labels.astype(np.int64)]
    lse = np.log(s.astype(np.float64))
    nll = lse - tgt.astype(np.float64)
    w64 = lw.astype(np.float64)
    loss = (w64 * nll).sum() / max(w64.sum(), 1.0)
    return np.float32(loss)



# revision 23
# speedup vs baseline: 1.8665x; 1.8665x over previous
"""Fused linear + cross-entropy loss (BaseChunkLoss) on 8 trn2 NeuronCores.

Hybrid sharding (2-way tokens x 4-way vocab, per the hint's tensor-parallel
option): core c = (a, b) with a = c // 4 (token half), b = c % 4 (vocab
quarter) handles 4096 tokens x 8000 vocab. Each core reads only W[:, b-slice]
(65.5 MB) and h[:, a-half] (33.5 MB) -- ~100 MB/core vs 270 MB for pure
token sharding -- so DMA (~344 us modeled) drops below the fp8 PE roofline
(~427 us) and the kernel becomes tensor-engine-bound.

Device layout: tokens on PSUM partitions, vocab on the free dim.
  stationary lhsT = hidden^T tile [128 d x (2 x 128 tok)] fp8
  moving rhs      = weight^T tile [128 d x (2 x 500 vocab)] fp8
Matmuls in fp8e4m3 DoubleRow (K=256/pass, 0.5 cyc/col); W pre-scaled by 64
on-chip for e4m3 range. Per (m-block, 1000-vocab group): DVE rewrites the
psum in place as logits+bias via (psum/64 + bias), then ACT computes exp
with a fused row-sum accumulator -> s partial per token. fp32->fp8
conversions are spread across Pool(gpsimd)/ACT/DVE so no engine exceeds the
PE bound.

Cross-device reduction (the wrapper's all_reduce): host sums the per-core
partial exp-sums over the 4 vocab shards, takes log, and combines with the
exact target logit. The target logit h . W[label] is computed on device as
an fp32 rowdot: each token's label lives in exactly one vocab quarter, so
the host routes (h row, W[label] row) pairs to the owning core, padded to a
fixed 1280 rows (actual counts ~1024 +- 60).

Host-side input prep is layout-only (transpose/slice/gather); all FLOPs
over hidden/weights happen on device inside the measured kernel.
"""
import numpy as np
from contextlib import ExitStack

from concourse import bacc, mybir, tile
from concourse.bass_utils import run_bass_kernel_spmd

F32 = mybir.dt.float32
BF16 = mybir.dt.bfloat16
FP8 = mybir.dt.float8e5
Alu = mybir.AluOpType
Act = mybir.ActivationFunctionType

N_CORES = 8
N_TOK = 8192
D = 2048
V = 32000
P = 128

A_SHARD = 2            # token shards
B_SHARD = 4            # vocab shards
T = N_TOK // A_SHARD   # 4096 tokens per core
VC = V // B_SHARD      # 8000 vocab per core

KP2 = D // 256         # 8 DoubleRow passes of K=256
BANK = 500             # vocab columns per psum bank (<= 512 fp32)
NB = 2                 # banks per vocab group
GV = NB * BANK         # 1000 vocab per group
NG = VC // GV          # 8 groups per core
MB = T // P            # 32 token blocks per core

TPAD = 1280            # padded rowdot rows per core (actual max 1074)
RB = TPAD // P         # 10 rowdot blocks

HQ = 512               # h staging chunk (tokens per DMA)
AG = 3                 # groups computed in the staggered token-chunk phase A
HS_BUFS = 4            # hstage ring depth
WS_BUFS = 3            # wstage ring depth
CONV_SPLIT = False     # phase-A W conversions split DVE/ACT/Pool vs all-Pool
INTERLEAVE_W = False   # interleave W chunks with h chunks inside a window
ET_SBUF = True         # descale writes bf16 logits to SBUF; exp reads there


def _build():
    nc = bacc.Bacc("TRN2", target_bir_lowering=False, debug=False)
    h_d = nc.declare_dram_parameter("h", [D, T], F32, isOutput=False)
    W_d = nc.declare_dram_parameter("W", [D, VC], F32, isOutput=False)
    bias_d = nc.declare_dram_parameter("bias", [VC], F32, isOutput=False)
    hn_d = nc.declare_dram_parameter("hn", [TPAD, D], F32, isOutput=False)
    wg_d = nc.declare_dram_parameter("wg", [TPAD, D], F32, isOutput=False)
    s_out = nc.declare_dram_parameter("s_out", [P, MB], F32, isOutput=True)
    t_out = nc.declare_dram_parameter("t_out", [P, RB], F32, isOutput=True)

    # d = kp*256 + j*128 + ki : row pairs (d, d+128) share a partition, as
    # DoubleRow consumes them from the j free dim.
    W_r2 = W_d[:].rearrange("(kp j ki) v -> kp ki j v", ki=P, j=2)
    h_r2 = h_d[:].rearrange("(kp j ki) t -> kp ki j t", ki=P, j=2)

    with tile.TileContext(nc) as tc, ExitStack() as ctx:
        hpool = ctx.enter_context(tc.tile_pool(name="hT", bufs=1))
        hstage = ctx.enter_context(tc.tile_pool(name="hstage", bufs=HS_BUFS))
        wpool = ctx.enter_context(tc.tile_pool(name="w", bufs=AG + 1))
        wstage = ctx.enter_context(tc.tile_pool(name="wstage", bufs=WS_BUFS))
        bpool = ctx.enter_context(tc.tile_pool(name="bias", bufs=1))
        bstage = ctx.enter_context(tc.tile_pool(name="bstage", bufs=1))
        rpool = ctx.enter_context(tc.tile_pool(name="rowdot", bufs=1))
        epool = (ctx.enter_context(tc.tile_pool(name="et", bufs=3))
                 if ET_SBUF else None)
        djunk = ctx.enter_context(tc.tile_pool(name="djunk", bufs=1))
        pspool = ctx.enter_context(tc.tile_pool(name="ps", bufs=4, space="PSUM"))
        acc = ctx.enter_context(tc.tile_pool(name="acc", bufs=1))

        s_cols = acc.tile([P, MB * NG], F32, tag="scols")
        t_fin = acc.tile([P, RB], F32, tag="tfin")
        t_half = acc.tile([P, 2 * RB], F32, tag="thalf")

        # Pre-warm the Exp table so the 1.3us LoadActFuncSet overlaps the
        # initial DMA lead-in instead of the first psum drain.
        warm = acc.tile([P, 2], F32, tag="warm")
        nc.vector.memset(warm[:], 0.0)
        nc.scalar.activation(warm[:], warm[:], Act.Exp)

        hT = hpool.tile([P, KP2, 2, T], FP8, tag="hT")

        def load_h(kp, tq):
            # h copies split Pool/DVE/ACT so no single engine paces the
            # h-streaming phase.
            st = hstage.tile([P, 2, HQ], F32, tag="hstage")
            nc.sync.dma_start(st[:], h_r2[kp][:, :, tq * HQ:(tq + 1) * HQ])
            dst = hT[:, kp, :, tq * HQ:(tq + 1) * HQ]
            if kp < 3:
                nc.gpsimd.tensor_copy(dst, st[:])
            elif kp < 6:
                nc.vector.tensor_copy(dst, st[:])
            else:
                nc.scalar.copy(dst, st[:])

        wtiles = {}

        def load_w_chunk(g, kp):
            if g not in wtiles:
                wtiles[g] = wpool.tile([P, KP2, 2, GV], FP8, tag="w", name=f"wv{g}")
            ws = wstage.tile([P, 2, GV], F32, tag="wstage")
            nc.sync.dma_start(ws[:], W_r2[kp][:, :, g * GV:(g + 1) * GV])
            if CONV_SPLIT and g <= AG and kp < 3:
                nc.vector.tensor_copy(wtiles[g][:, kp], ws[:])
            elif CONV_SPLIT and g <= AG and kp < 6:
                nc.scalar.copy(wtiles[g][:, kp], ws[:])
            else:
                nc.gpsimd.tensor_copy(wtiles[g][:, kp], ws[:])

        btiles = {}

        def load_bias(g):
            btiles[g] = bpool.tile([P, GV], BF16, tag=f"bias{g}", name=f"bias{g}")
            bs = bstage.tile([P, GV], F32, tag="bstage")
            nc.sync.dma_start(
                bs[:], bias_d[g * GV:(g + 1) * GV].partition_broadcast(P))
            nc.vector.tensor_copy(btiles[g][:], bs[:])

        DH = D // 2

        def rowdot(r):
            for half in range(2):
                dsl = slice(half * DH, (half + 1) * DH)
                hg = rpool.tile([P, DH], F32, tag="hg")
                nc.sync.dma_start(hg[:], hn_d[r * P:(r + 1) * P, dsl])
                wgt = rpool.tile([P, DH], F32, tag="wgt")
                nc.sync.dma_start(wgt[:], wg_d[r * P:(r + 1) * P, dsl])
                dj = djunk.tile([P, DH], BF16, tag="djunk")
                nc.vector.scalar_tensor_tensor(
                    dj[:], hg[:], 1.0, wgt[:], op0=Alu.mult, op1=Alu.mult,
                    accum_out=t_half[:, 2 * r + half:2 * r + half + 1])

        def tile_iter(g, m):
            pt = pspool.tile([P, NB, 512], F32, tag="ps")
            wv = wtiles[g]
            for kp in range(KP2):
                lhsT = hT[:, kp, :, m * P:(m + 1) * P]
                for bk in range(NB):
                    nc.tensor.matmul(
                        pt[:, bk, 0:BANK], lhsT,
                        wv[:, kp, :, bk * BANK:(bk + 1) * BANK],
                        start=(kp == 0), stop=(kp == KP2 - 1),
                        perf_mode=mybir.MatmulPerfMode.DoubleRow,
                    )
            psl = pt[:, :, 0:BANK]
            bbv = btiles[g][:].rearrange("p (b c) -> p b c", c=BANK)
            if ET_SBUF:
                # psum bank frees after the DVE pass; exp drains from SBUF
                et = epool.tile([P, NB, BANK], BF16, tag="et")
                nc.vector.tensor_tensor(et[:], psl, bbv, op=Alu.add)
                nc.scalar.activation(
                    et[:], et[:], Act.Exp,
                    accum_out=s_cols[:, m * NG + g:m * NG + g + 1])
            else:
                nc.vector.tensor_tensor(psl, psl, bbv, op=Alu.add)
                nc.scalar.activation(
                    psl, psl, Act.Exp,
                    accum_out=s_cols[:, m * NG + g:m * NG + g + 1])

        # --- phase A: staggered token-chunk-major. Group g's compute starts
        # at window g (right after its W group has streamed in), so W arrival
        # and compute order match. Window w: h tq w arrives; W group w lands
        # whole (w <= AG); group g computes m-blocks of tq (w - g).
        NTQ = T // HQ
        MQ = HQ // P           # m-blocks per h chunk
        for w in range(NTQ + AG - 1):
            if w < NTQ:
                for kp in range(KP2):
                    load_h(kp, w)
                    if w <= AG and (INTERLEAVE_W or w == 0):
                        load_w_chunk(w, kp)
                if w <= AG:
                    if not (INTERLEAVE_W or w == 0):
                        for kp in range(KP2):
                            load_w_chunk(w, kp)
                    load_bias(w)
            for g in range(AG):
                tq = w - g
                if 0 <= tq < NTQ:
                    for m in range(MQ * tq, MQ * (tq + 1)):
                        tile_iter(g, m)

        # --- phase B: groups AG..NG-1, one W group prefetched ahead ---
        for g in range(AG, NG):
            if g + 1 < NG:
                load_bias(g + 1)
                for kp in range(KP2):
                    load_w_chunk(g + 1, kp)
            r0 = (RB * (g - AG)) // (NG - AG)
            r1 = (RB * (g - AG + 1)) // (NG - AG)
            for r in range(r0, r1):
                rowdot(r)
            for m in range(MB):
                tile_iter(g, m)

        s_fin = acc.tile([P, MB], F32, tag="sfin")
        for m in range(MB):
            nc.vector.tensor_reduce(
                s_fin[:, m:m + 1], s_cols[:, m * NG:(m + 1) * NG],
                axis=mybir.AxisListType.X, op=Alu.add)
        for r in range(RB):
            nc.vector.tensor_tensor(
                t_fin[:, r:r + 1], t_half[:, 2 * r:2 * r + 1],
                t_half[:, 2 * r + 1:2 * r + 2], op=Alu.add)
        nc.sync.dma_start(s_out[:], s_fin[:])
        nc.sync.dma_start(t_out[:], t_fin[:])

    nc.compile()
    return nc


_NC_CACHE = {}


def _get_program():
    if "nc" not in _NC_CACHE:
        _NC_CACHE["nc"] = _build()
    return _NC_CACHE["nc"]


def kernel(hidden_states, head_weight, head_bias, loss_weight, labels,
           chunk_size=None, **_unused):
    hidden = np.asarray(hidden_states, dtype=np.float32)
    W = np.asarray(head_weight, dtype=np.float32)
    bias = np.asarray(head_bias, dtype=np.float32)
    lw = np.asarray(loss_weight, dtype=np.float32)
    labels = np.asarray(labels).astype(np.int64)

    assert hidden.shape == (N_TOK, D) and W.shape == (V, D)

    nc = _get_program()
    Wt = np.ascontiguousarray(W.T)                 # [D, V]
    ht = np.ascontiguousarray(hidden.T)            # [D, N]

    in_maps = []
    core_idx = []                                  # rowdot token indices
    for c in range(N_CORES):
        a, b = c // B_SHARD, c % B_SHARD
        tsl = slice(a * T, (a + 1) * T)
        vlo = b * VC
        lab_c = labels[tsl]
        idx = np.nonzero((lab_c >= vlo) & (lab_c < vlo + VC))[0]
        assert len(idx) <= TPAD, f"core {c}: {len(idx)} rowdot rows > {TPAD}"
        core_idx.append(idx)
        hn = np.zeros((TPAD, D), dtype=np.float32)
        hn[:len(idx)] = hidden[tsl][idx]
        wg = np.zeros((TPAD, D), dtype=np.float32)
        wg[:len(idx)] = W[lab_c[idx]]
        in_maps.append(dict(
            h=np.ascontiguousarray(ht[:, tsl]),
            W=np.ascontiguousarray(Wt[:, vlo:vlo + VC]),
            bias=np.ascontiguousarray(bias[vlo:vlo + VC]),
            hn=hn, wg=wg))

    res = run_bass_kernel_spmd(nc, in_maps, list(range(N_CORES)))

    # unshard + host-side combine (the "all_reduce" of the hint)
    s = np.zeros(N_TOK, dtype=np.float64)
    tgt = np.zeros(N_TOK, dtype=np.float64)
    for c in range(N_CORES):
        a = c // B_SHARD
        tsl = slice(a * T, (a + 1) * T)
        # token t = m*128 + p  ->  s_out[p, m]
        s[tsl] += res.results[c]["s_out"].T.reshape(-1).astype(np.float64)
        idx = core_idx[c]
        td = res.results[c]["t_out"].T.reshape(-1)[:len(idx)]
        tgt[a * T + idx] = td
    tgt = tgt + bias[labels].astype(np.float64)

    lse = np.log(s)
    nll = lse - tgt
    w64 = lw.astype(np.float64)
    loss = (w64 * nll).sum() / max(w64.sum(), 1.0)
    return np.float32(loss)
